# revision 6
# baseline (speedup 1.0000x reference)
"""GCN JetTagger on 8 trn2 NeuronCores (Bass/Tile).

kernel(**inputs) takes FULL inputs (as reference.setup_inputs()) and returns
(label_out [512,1], dom_out [512,2]) matching reference.reference(**inputs).

Sharding: graphs 64/core (batch is sorted -> contiguous node ranges).
Edges partitioned by dst-owning core, dst-sorted, tiled into 128-dst tiles.
Gather of source-node rows via dma_gather (int16 idx -> table split in two
halves at row 32768). Aggregation = per-128-edge-chunk indicator matmuls
(PE) accumulating into PSUM. W-matmuls folded: table1 = dinv*(x@W1);
table2 = dinv*relu(dinv*agg1 + b1) allgathered across cores;
h2 = relu((dinv*agg2)@W2 + b2); mean-pool via segment-indicator matmul.
"""
import contextlib

import numpy as np
import ml_dtypes

N_CORES = 8
N, E, G = 50000, 1600000, 512
IN, H = 16, 128
NT = 392                 # global node tiles (NFULL = 50176)
NFULL = NT * 128
TLOC = 52                # local node tiles per core
NP = TLOC * 128          # padded nodes per core = 6656
GPC = G // N_CORES       # graphs per core = 64
HALF = 32768             # int16 index limit -> table split
# per (dst-tile, table-half) gather-call schedule (statically shared by cores)
CALLS_A = [1024, 1024, 1024]   # cap 3072 edges with src-row < HALF
CALLS_B = [1024, 768]          # cap 1792 edges with src-row >= HALF
CAP_A = sum(CALLS_A)
CAP_B = sum(CALLS_B)
CAP_T = CAP_A + CAP_B          # 4864 edges per tile
ECAP = TLOC * CAP_T            # per-core padded edge stream
NCHUNK = ECAP // 128           # indicator chunks
ST = ECAP // 16                # idx stream columns

bf16 = ml_dtypes.bfloat16

_cache = {}


def _build_module():
    import concourse.bacc as bacc
    import concourse.mybir as mybir
    import concourse.tile as tile
    from concourse.masks import make_identity

    f32 = mybir.dt.float32
    bf = mybir.dt.bfloat16
    i16 = mybir.dt.int16
    AF = mybir.ActivationFunctionType
    ALU = mybir.AluOpType

    nc = bacc.Bacc("TRN2", target_bir_lowering=False, debug=False,
                   num_devices=N_CORES, num_swdge_queues=4)

    # ---- inputs ----
    din = {}
    def inp(name, shape, dt=f32):
        din[name] = nc.dram_tensor(name, shape, dt, kind="ExternalInput")
        return din[name]

    xT_d = inp("xT", [IN, NFULL])
    xTl_d = inp("xTl", [IN, NP])
    deg_d = inp("deg", [NFULL])
    degl_d = inp("degl", [NP])
    W1_d = inp("W1i", [IN, H])
    W2_d = inp("W2i", [H, H])
    Wlab_d = inp("Wlabi", [H, 1])
    Wd1_d = inp("Wd1i", [H, 64])
    Wd2_d = inp("Wd2i", [64, 2])
    b1t_d = inp("b1t", [128, H])
    b2c_d = inp("b2c", [H, 1])
    blab_d = inp("blabc", [1, 1])
    bd1_d = inp("bd1c", [64, 1])
    bd2_d = inp("bd2c", [2, 1])
    idx1_d = inp("idx1", [128, ST], i16)
    idx2_d = inp("idx2", [128, ST], i16)
    dl1_d = inp("dl1", [128, NCHUNK], bf)
    dl2_d = inp("dl2", [128, NCHUNK], bf)
    seg_d = inp("segind", [128, TLOC * GPC], bf)
    cnt_d = inp("cnts", [GPC, 1])
    iota_d = inp("iota", [128, 128], bf)

    lab_d = nc.dram_tensor("lab", [1, GPC], f32, kind="ExternalOutput")
    dom_d = nc.dram_tensor("dom", [2, GPC], f32, kind="ExternalOutput")

    table1_d = nc.dram_tensor("table1", [NFULL, H], bf)
    shard_d = nc.dram_tensor("shard", [NP, H], bf)
    t2full_d = nc.dram_tensor("t2full", [N_CORES * NP, H], bf,
                              addr_space="Shared")

    FULL2 = N_CORES * NP

    with tile.TileContext(nc) as tc:
        with contextlib.ExitStack() as ctx:
            const = ctx.enter_context(tc.tile_pool(name="const", bufs=1))
            work = ctx.enter_context(tc.tile_pool(name="work", bufs=3))
            gpool = ctx.enter_context(tc.tile_pool(name="gp", bufs=4))
            ipool = ctx.enter_context(tc.tile_pool(name="ip", bufs=4))
            ppool = ctx.enter_context(tc.tile_pool(name="ps", bufs=2, space="PSUM"))

            # ---- constants to SBUF ----
            def load_const(dram, shape, dt, tag):
                t = const.tile(shape, dt, tag=tag)
                nc.sync.dma_start(out=t[:], in_=dram.ap())
                return t

            iota_t = load_const(iota_d, [128, 128], bf, "c_iota")
            b1t = load_const(b1t_d, [128, H], f32, "c_b1t")
            b2c = load_const(b2c_d, [H, 1], f32, "c_b2c")
            blabc = load_const(blab_d, [1, 1], f32, "c_blab")
            bd1c = load_const(bd1_d, [64, 1], f32, "c_bd1")
            bd2c = load_const(bd2_d, [2, 1], f32, "c_bd2")
            Wlab = load_const(Wlab_d, [H, 1], f32, "c_wlab")
            Wd1 = load_const(Wd1_d, [H, 64], f32, "c_wd1")
            Wd2 = load_const(Wd2_d, [64, 2], f32, "c_wd2")
            cnts = load_const(cnt_d, [GPC, 1], f32, "c_cnt")
            seg_t = load_const(seg_d, [128, TLOC * GPC], bf, "c_seg")
            dl1 = load_const(dl1_d, [128, NCHUNK], bf, "c_dl1")
            dl2 = load_const(dl2_d, [128, NCHUNK], bf, "c_dl2")

            idf = const.tile([128, 128], f32)
            make_identity(nc, idf[:])
            idb = const.tile([128, 128], bf)
            nc.vector.tensor_copy(out=idb[:], in_=idf[:])

            # W1, W2 cast to bf16
            W1f = const.tile([IN, H], f32)
            nc.sync.dma_start(out=W1f[:], in_=W1_d.ap())
            W2f = const.tile([H, H], f32)
            nc.sync.dma_start(out=W2f[:], in_=W2_d.ap())
            W2b = const.tile([H, H], bf)
            nc.vector.tensor_copy(out=W2b[:], in_=W2f[:])


            # dinv = 1/sqrt(deg): sqrt on ACT, reciprocal on DVE
            dinv = const.tile([128, NT], f32)
            degt = const.tile([128, NT], f32)
            nc.sync.dma_start(out=degt[:],
                              in_=deg_d.ap().rearrange("(t p) -> p t", p=128))
            nc.scalar.activation(out=dinv[:], in_=degt[:], func=AF.Sqrt)
            nc.vector.reciprocal(out=dinv[:], in_=dinv[:])
            dinvl = const.tile([128, TLOC], f32)
            deglt = const.tile([128, TLOC], f32)
            nc.sync.dma_start(out=deglt[:],
                              in_=degl_d.ap().rearrange("(t p) -> p t", p=128))
            nc.scalar.activation(out=dinvl[:], in_=deglt[:], func=AF.Sqrt)
            nc.vector.reciprocal(out=dinvl[:], in_=dinvl[:])

            # ---- phase 1: table1 = dinv * (x @ W1), full graph ----
            for t in range(NT):
                xt = work.tile([IN, 128], f32, tag="xt")
                nc.sync.dma_start(out=xt[:],
                                  in_=xT_d.ap()[:, t * 128:(t + 1) * 128])
                ps = ppool.tile([128, H], f32, tag="post")
                nc.tensor.matmul(out=ps[:], lhsT=xt[:],
                                 rhs=W1f[:], start=True, stop=True)
                sb = work.tile([128, H], bf, tag="tb_sb")
                nc.scalar.activation(out=sb[:], in_=ps[:], func=AF.Copy,
                                     scale=dinv[:, t:t + 1])
                nc.sync.dma_start(out=table1_d.ap()[t * 128:(t + 1) * 128, :],
                                  in_=sb[:])

            # local table1 rows (self-loop contributions), kept in SBUF
            t1loc = const.tile([128, TLOC * H], bf)
            for t in range(TLOC):
                xt = work.tile([IN, 128], f32, tag="xt")
                nc.sync.dma_start(out=xt[:],
                                  in_=xTl_d.ap()[:, t * 128:(t + 1) * 128])
                ps = ppool.tile([128, H], f32, tag="post")
                nc.tensor.matmul(out=ps[:], lhsT=xt[:],
                                 rhs=W1f[:], start=True, stop=True)
                nc.scalar.activation(out=t1loc[:, t * H:(t + 1) * H], in_=ps[:],
                                     func=AF.Copy, scale=dinvl[:, t:t + 1])

            # ---- generic aggregation pass ----
            def agg_pass(idx_dram, dl_tile, tableA_ap, tableB_ap, self_rows,
                         post):
                """for each local dst tile: psum = sum over edges of
                table[src]; then post(t, psum)."""
                call_no = [0]
                for t in range(TLOC):
                    psum = ppool.tile([128, H], f32, tag="agg")
                    chunk0 = t * (CAP_T // 128)
                    nch = CAP_T // 128
                    ci = 0
                    off = t * CAP_T
                    for half, calls in ((0, CALLS_A), (1, CALLS_B)):
                        src_ap = tableA_ap if half == 0 else tableB_ap
                        for n in calls:
                            it = ipool.tile([128, n // 16], i16, tag="idx")
                            nc.sync.dma_start(
                                out=it[:],
                                in_=idx_dram.ap()[:, off // 16:(off + n) // 16])
                            g = gpool.tile([128, n // 128, H], bf, tag="g")
                            if call_no[0] < 4:
                                nc.vector.memset(g[:], 0.0)
                            call_no[0] += 1
                            nc.gpsimd.dma_gather(
                                out_ap=g[:], in_ap=src_ap, idxs_ap=it[:],
                                num_idxs=n, num_idxs_reg=n, elem_size=H,
                                queue_num=(t + ci) % 4)
                            for j in range(n // 128):
                                ind = ipool.tile([128, 128], bf, tag="ind")
                                nc.vector.tensor_tensor(
                                    out=ind[:],
                                    in0=dl_tile[:, chunk0 + ci:chunk0 + ci + 1]
                                        .to_broadcast([128, 128]),
                                    in1=iota_t[:], op=ALU.is_equal)
                                nc.tensor.matmul(
                                    out=psum[:], lhsT=ind[:], rhs=g[:, j, :],
                                    start=(ci == 0), stop=(ci == nch - 1))
                                ci += 1
                            off += n
                    post(t, psum, self_rows(t))

            # ---- phase 2: layer-1 aggregation -> table2 shard ----
            t2loc = const.tile([128, TLOC * H], bf)

            def l1_post(t, psum, selfr):
                s1 = work.tile([128, H], f32, tag="s1")
                nc.vector.tensor_tensor(out=s1[:], in0=psum[:], in1=selfr,
                                        op=ALU.add)
                nc.vector.tensor_tensor(
                    out=s1[:], in0=s1[:],
                    in1=dinvl[:, t:t + 1].to_broadcast([128, H]), op=ALU.mult)
                nc.vector.tensor_tensor(out=s1[:], in0=s1[:], in1=b1t[:],
                                        op=ALU.add)
                nc.scalar.activation(out=t2loc[:, t * H:(t + 1) * H], in_=s1[:],
                                     func=AF.Relu, scale=dinvl[:, t:t + 1])
                nc.sync.dma_start(out=shard_d.ap()[t * 128:(t + 1) * 128, :],
                                  in_=t2loc[:, t * H:(t + 1) * H])

            agg_pass(idx1_d, dl1,
                     table1_d.ap()[0:HALF, :], table1_d.ap()[HALF:NFULL, :],
                     lambda t: t1loc[:, t * H:(t + 1) * H], l1_post)

            # ---- phase 3: allgather table2 ----
            nc.gpsimd.collective_compute(
                "AllGather", ALU.bypass,
                replica_groups=[list(range(N_CORES))],
                ins=[shard_d.ap()], outs=[t2full_d.ap()])

            # ---- phase 4: layer-2 aggregation -> u2 (node-major) -> u2_fm ----
            u2fm = const.tile([128, NP], bf)

            def l2_post(t, psum, selfr):
                s1 = work.tile([128, H], bf, tag="s2")
                nc.vector.tensor_tensor(out=s1[:], in0=psum[:], in1=selfr,
                                        op=ALU.add)
                nc.vector.tensor_tensor(
                    out=s1[:], in0=s1[:],
                    in1=dinvl[:, t:t + 1].to_broadcast([128, H]), op=ALU.mult)
                ptr = ppool.tile([128, 128], bf, tag="ptb")
                nc.tensor.transpose(out=ptr[:], in_=s1[:], identity=idb[:])
                nc.vector.tensor_copy(out=u2fm[:, t * 128:(t + 1) * 128],
                                      in_=ptr[:])

            agg_pass(idx2_d, dl2,
                     t2full_d.ap()[0:HALF, :], t2full_d.ap()[HALF:FULL2, :],
                     lambda t: t2loc[:, t * H:(t + 1) * H], l2_post)

            # ---- phase 5: h2 = relu(u2 @ W2 + b2) (feature-major) ----
            h2fm = const.tile([128, NP], bf)
            for k in range(NP // 512):
                ps = ppool.tile([128, 512], f32, tag="post")
                nc.tensor.matmul(out=ps[:], lhsT=W2b[:],
                                 rhs=u2fm[:, k * 512:(k + 1) * 512],
                                 start=True, stop=True)
                nc.scalar.activation(out=h2fm[:, k * 512:(k + 1) * 512],
                                     in_=ps[:], func=AF.Relu, bias=b2c[:])

            # ---- phase 6: transpose h2 back, pool via segment matmul ----
            h2nm = const.tile([128, NP], bf)
            for t in range(TLOC):
                ptr = ppool.tile([128, 128], bf, tag="ptb")
                nc.tensor.transpose(out=ptr[:],
                                    in_=h2fm[:, t * 128:(t + 1) * 128],
                                    identity=idb[:])
                nc.vector.tensor_copy(out=h2nm[:, t * 128:(t + 1) * 128],
                                      in_=ptr[:])

            pps = ppool.tile([GPC, H], f32, tag="pool")
            for t in range(TLOC):
                nc.tensor.matmul(out=pps[:],
                                 lhsT=seg_t[:, t * GPC:(t + 1) * GPC],
                                 rhs=h2nm[:, t * 128:(t + 1) * 128],
                                 start=(t == 0), stop=(t == TLOC - 1))
            rc = work.tile([GPC, 1], f32, tag="rc")
            nc.vector.reciprocal(out=rc[:], in_=cnts[:])
            poolnm = work.tile([GPC, H], f32, tag="poolnm")
            nc.scalar.activation(out=poolnm[:], in_=pps[:], func=AF.Copy,
                                 scale=rc[:])
            ptr = ppool.tile([128, GPC], f32, tag="post")
            nc.tensor.transpose(out=ptr[:, :GPC], in_=poolnm[:],
                                identity=idf[:GPC, :GPC])
            poolfm = work.tile([H, GPC], f32, tag="poolfm")
            nc.vector.tensor_copy(out=poolfm[:], in_=ptr[:, :GPC])

            # ---- phase 7: heads ----
            psl = ppool.tile([1, GPC], f32, tag="post")
            nc.tensor.matmul(out=psl[:], lhsT=Wlab[:], rhs=poolfm[:],
                             start=True, stop=True)
            labt = work.tile([1, GPC], f32, tag="lab")
            nc.scalar.activation(out=labt[:], in_=psl[:], func=AF.Sigmoid,
                                 bias=blabc[:])
            nc.sync.dma_start(out=lab_d.ap(), in_=labt[:])

            psd1 = ppool.tile([64, GPC], f32, tag="post")
            nc.tensor.matmul(out=psd1[:], lhsT=Wd1[:], rhs=poolfm[:],
                             start=True, stop=True)
            d1t = work.tile([64, GPC], f32, tag="d1")
            nc.scalar.activation(out=d1t[:], in_=psd1[:], func=AF.Relu,
                                 bias=bd1c[:])
            psd2 = ppool.tile([2, GPC], f32, tag="post")
            nc.tensor.matmul(out=psd2[:], lhsT=Wd2[:], rhs=d1t[:],
                             start=True, stop=True)
            domt = work.tile([2, GPC], f32, tag="dm")
            nc.scalar.activation(out=domt[:], in_=psd2[:], func=AF.Identity,
                                 bias=bd2c[:])
            nc.sync.dma_start(out=dom_d.ap(), in_=domt[:])

    nc.compile()
    return nc


def _prep_inputs(x, edge_index, batch, W1, b1, W2, b2, Wlab, blab, Wd1, bd1,
                 Wd2, bd2, sim_pads=True):
    """Host-side sharding/layout: pure index bucketing + layout transforms."""
    x = np.asarray(x, np.float32)
    ei = np.asarray(edge_index, np.int64)
    batch = np.asarray(batch, np.int64)

    # graph -> core, node ranges (batch sorted)
    nlo = np.searchsorted(batch, np.arange(0, G, GPC))
    nhi = np.append(nlo[1:], N)
    owner = np.zeros(N, np.int64)
    for c in range(N_CORES):
        owner[nlo[c]:nhi[c]] = c
    assert (nhi - nlo).max() <= NP, f"core node count {(nhi-nlo).max()} > {NP}"

    src = np.concatenate([ei[0], np.arange(N, dtype=np.int64)])
    dst = np.concatenate([ei[1], np.arange(N, dtype=np.int64)])
    is_self = np.zeros(len(src), bool)
    is_self[E:] = True

    # degrees at dst (incl self-loops) -- sharding metadata
    deg = np.bincount(dst, minlength=N).astype(np.float32)
    deg_full = np.ones(NFULL, np.float32)
    deg_full[:N] = deg

    # drop self-loops from the gathered stream (handled via local row adds)
    m = ~is_self
    src, dst = src[m], dst[m]

    row2 = owner[src] * NP + (src - nlo[owner[src]])  # layer-2 table row ids

    per_core = []
    for c in range(N_CORES):
        sel = (dst >= nlo[c]) & (dst < nhi[c])
        s_g = src[sel]
        d_l = (dst[sel] - nlo[c]).astype(np.int64)
        r2 = row2[sel]
        order = np.argsort(d_l, kind="stable")
        s_g, d_l, r2 = s_g[order], d_l[order], r2[order]

        def build_stream(rows):
            idx_stream = np.zeros(ECAP, np.int16) if sim_pads else \
                np.full(ECAP, -1, np.int16)
            dl_stream = np.full(ECAP, -1.0, np.float32)
            tile_of = d_l // 128
            for t in range(TLOC):
                smask = tile_of == t
                rt = rows[smask]
                dt_ = d_l[smask] - t * 128
                a = rt < HALF
                ra, da = rt[a], dt_[a]
                rb, db = rt[~a] - HALF, dt_[~a]
                assert len(ra) <= CAP_A, f"tile {t} A overflow {len(ra)}"
                assert len(rb) <= CAP_B, f"tile {t} B overflow {len(rb)}"
                o = t * CAP_T
                idx_stream[o:o + len(ra)] = ra.astype(np.int16)
                dl_stream[o:o + len(ra)] = da
                o2 = o + CAP_A
                idx_stream[o2:o2 + len(rb)] = rb.astype(np.int16)
                dl_stream[o2:o2 + len(rb)] = db
            idxp = np.tile(idx_stream.reshape(ST, 16).T, (8, 1)).copy()
            dlp = np.ascontiguousarray(
                dl_stream.reshape(NCHUNK, 128).T).astype(bf16)
            return idxp, dlp

        idx1, dl1 = build_stream(s_g)
        idx2, dl2 = build_stream(r2)

        nodes_c = nhi[c] - nlo[c]
        xTl = np.zeros((IN, NP), np.float32)
        xTl[:, :nodes_c] = x[nlo[c]:nhi[c]].T
        degl = np.ones(NP, np.float32)
        degl[:nodes_c] = deg[nlo[c]:nhi[c]]

        segind = np.zeros((128, TLOC * GPC), bf16)
        bl = batch[nlo[c]:nhi[c]] - c * GPC
        node_ids = np.arange(nodes_c)
        segind_full = np.zeros((NP, GPC), np.float32)
        segind_full[node_ids, bl] = 1.0
        for t in range(TLOC):
            segind[:, t * GPC:(t + 1) * GPC] = \
                segind_full[t * 128:(t + 1) * 128].astype(bf16)
        cnts = np.maximum(
            np.bincount(bl, minlength=GPC), 1).astype(np.float32)[:, None]

        per_core.append(dict(
            xTl=xTl, degl=degl, idx1=idx1, idx2=idx2, dl1=dl1, dl2=dl2,
            segind=segind, cnts=cnts))

    xT = np.zeros((IN, NFULL), np.float32)
    xT[:, :N] = x.T
    shared = dict(
        xT=xT, deg=deg_full,
        W1i=np.asarray(W1, np.float32), W2i=np.asarray(W2, np.float32),
        Wlabi=np.asarray(Wlab, np.float32),
        Wd1i=np.asarray(Wd1, np.float32), Wd2i=np.asarray(Wd2, np.float32),
        b1t=np.tile(np.asarray(b1, np.float32)[None, :], (128, 1)),
        b2c=np.asarray(b2, np.float32)[:, None],
        blabc=np.asarray(blab, np.float32)[:, None],
        bd1c=np.asarray(bd1, np.float32)[:, None],
        bd2c=np.asarray(bd2, np.float32)[:, None],
        iota=np.tile(np.arange(128, dtype=np.float32)[None, :],
                     (128, 1)).astype(bf16),
    )
    in_maps = [{**shared, **pc} for pc in per_core]
    return in_maps


def kernel(x, edge_index, batch, W1, b1, W2, b2, Wlab, blab, Wd1, bd1, Wd2,
           bd2):
    if "nc" not in _cache:
        _cache["nc"] = _build_module()
    nc = _cache["nc"]
    in_maps = _prep_inputs(x, edge_index, batch, W1, b1, W2, b2, Wlab, blab,
                           Wd1, bd1, Wd2, bd2)
    from concourse.bass_utils import run_bass_kernel_spmd
    res = run_bass_kernel_spmd(nc, in_maps, list(range(N_CORES)))
    labs, doms = [], []
    for c in range(N_CORES):
        labs.append(np.asarray(res.results[c]["lab"]).T)       # [64,1]
        doms.append(np.asarray(res.results[c]["dom"]).T)       # [64,2]
    label_out = np.concatenate(labs, axis=0).astype(np.float32)
    dom_out = np.concatenate(doms, axis=0).astype(np.float32)
    return label_out, dom_out


# revision 7
# speedup vs baseline: 2.2081x; 2.2081x over previous
"""GCN JetTagger on 8 trn2 NeuronCores (Bass/Tile).

kernel(**inputs) takes FULL inputs (as reference.setup_inputs()) and returns
(label_out [512,1], dom_out [512,2]) matching reference.reference(**inputs).

Sharding: graphs 64/core (batch is sorted -> contiguous node ranges).
Edges partitioned by dst-owning core, dst-sorted, tiled into 128-dst tiles.
Gather of source-node rows via dma_gather (int16 idx -> table split in two
halves at row 32768). Aggregation = per-128-edge-chunk indicator matmuls
(PE) accumulating into PSUM. W-matmuls folded: table1 = dinv*(x@W1);
table2 = dinv*relu(dinv*agg1 + b1) allgathered across cores;
h2 = relu((dinv*agg2)@W2 + b2); mean-pool via segment-indicator matmul.
"""
import contextlib

import numpy as np
import ml_dtypes

N_CORES = 8
N, E, G = 50000, 1600000, 512
IN, H = 16, 128
NT = 392                 # global node tiles (NFULL = 50176)
NFULL = NT * 128
TLOC = 52                # local node tiles per core
NP = TLOC * 128          # padded nodes per core = 6656
GPC = G // N_CORES       # graphs per core = 64
HALF = 32768             # int16 index limit -> table split
# per (dst-tile, table-half) gather-call schedule (statically shared by cores)
CALLS_A = [1024, 1024, 1024]   # cap 3072 edges with src-row < HALF
CALLS_B = [1024, 768]          # cap 1792 edges with src-row >= HALF
CAP_A = sum(CALLS_A)
CAP_B = sum(CALLS_B)
CAP_T = CAP_A + CAP_B          # 4864 edges per tile
ECAP = TLOC * CAP_T            # per-core padded edge stream
NCHUNK = ECAP // 128           # indicator chunks
ST = ECAP // 16                # idx stream columns

bf16 = ml_dtypes.bfloat16

_cache = {}


def _build_module():
    import concourse.bacc as bacc
    import concourse.mybir as mybir
    import concourse.tile as tile
    from concourse.masks import make_identity

    f32 = mybir.dt.float32
    bf = mybir.dt.bfloat16
    i16 = mybir.dt.int16
    AF = mybir.ActivationFunctionType
    ALU = mybir.AluOpType

    nc = bacc.Bacc("TRN2", target_bir_lowering=False, debug=False,
                   num_devices=N_CORES, num_swdge_queues=4)

    # ---- inputs ----
    din = {}
    def inp(name, shape, dt=f32):
        din[name] = nc.dram_tensor(name, shape, dt, kind="ExternalInput")
        return din[name]

    xT_d = inp("xT", [IN, NFULL])
    xTl_d = inp("xTl", [IN, NP])
    deg_d = inp("deg", [NFULL])
    degl_d = inp("degl", [NP])
    W1_d = inp("W1i", [IN, H])
    W2_d = inp("W2i", [H, H])
    Wlab_d = inp("Wlabi", [H, 1])
    Wd1_d = inp("Wd1i", [H, 64])
    Wd2_d = inp("Wd2i", [64, 2])
    b1t_d = inp("b1t", [128, H])
    b2c_d = inp("b2c", [H, 1])
    blab_d = inp("blabc", [1, 1])
    bd1_d = inp("bd1c", [64, 1])
    bd2_d = inp("bd2c", [2, 1])
    idx1_d = inp("idx1", [128, ST], i16)
    idx2_d = inp("idx2", [128, ST], i16)
    dl1_d = inp("dl1", [128, NCHUNK], bf)
    dl2_d = inp("dl2", [128, NCHUNK], bf)
    seg_d = inp("segind", [128, TLOC * GPC], bf)
    cnt_d = inp("cnts", [GPC, 1])
    iota_d = inp("iota", [128, 128], bf)

    lab_d = nc.dram_tensor("lab", [1, GPC], f32, kind="ExternalOutput")
    dom_d = nc.dram_tensor("dom", [2, GPC], f32, kind="ExternalOutput")

    table1_d = nc.dram_tensor("table1", [NFULL, H], bf)
    shard_d = nc.dram_tensor("shard", [NP, H], bf)
    t2full_d = nc.dram_tensor("t2full", [N_CORES * NP, H], bf,
                              addr_space="Shared")

    FULL2 = N_CORES * NP

    with tile.TileContext(nc) as tc:
        with contextlib.ExitStack() as ctx:
            const = ctx.enter_context(tc.tile_pool(name="const", bufs=1))
            work = ctx.enter_context(tc.tile_pool(name="work", bufs=3))
            gpool = ctx.enter_context(tc.tile_pool(name="gp", bufs=6))
            ipool = ctx.enter_context(tc.tile_pool(name="ip", bufs=6))
            ppool = ctx.enter_context(tc.tile_pool(name="ps", bufs=2, space="PSUM"))

            # ---- constants to SBUF ----
            def load_const(dram, shape, dt, tag):
                t = const.tile(shape, dt, tag=tag)
                nc.sync.dma_start(out=t[:], in_=dram.ap())
                return t

            iota_t = load_const(iota_d, [128, 128], bf, "c_iota")
            b1t = load_const(b1t_d, [128, H], f32, "c_b1t")
            b2c = load_const(b2c_d, [H, 1], f32, "c_b2c")
            blabc = load_const(blab_d, [1, 1], f32, "c_blab")
            bd1c = load_const(bd1_d, [64, 1], f32, "c_bd1")
            bd2c = load_const(bd2_d, [2, 1], f32, "c_bd2")
            Wlab = load_const(Wlab_d, [H, 1], f32, "c_wlab")
            Wd1 = load_const(Wd1_d, [H, 64], f32, "c_wd1")
            Wd2 = load_const(Wd2_d, [64, 2], f32, "c_wd2")
            cnts = load_const(cnt_d, [GPC, 1], f32, "c_cnt")
            seg_t = load_const(seg_d, [128, TLOC * GPC], bf, "c_seg")
            dl1 = load_const(dl1_d, [128, NCHUNK], bf, "c_dl1")
            dl2 = load_const(dl2_d, [128, NCHUNK], bf, "c_dl2")

            idf = const.tile([128, 128], f32)
            make_identity(nc, idf[:])
            idb = const.tile([128, 128], bf)
            nc.vector.tensor_copy(out=idb[:], in_=idf[:])

            # W1, W2 cast to bf16
            W1f = const.tile([IN, H], f32)
            nc.sync.dma_start(out=W1f[:], in_=W1_d.ap())
            W2f = const.tile([H, H], f32)
            nc.sync.dma_start(out=W2f[:], in_=W2_d.ap())
            W2b = const.tile([H, H], bf)
            nc.vector.tensor_copy(out=W2b[:], in_=W2f[:])


            # dinv = 1/sqrt(deg): sqrt on ACT, reciprocal on DVE
            dinv = const.tile([128, NT], f32)
            degt = const.tile([128, NT], f32)
            nc.sync.dma_start(out=degt[:],
                              in_=deg_d.ap().rearrange("(t p) -> p t", p=128))
            nc.scalar.activation(out=dinv[:], in_=degt[:], func=AF.Sqrt)
            nc.vector.reciprocal(out=dinv[:], in_=dinv[:])
            dinvl = const.tile([128, TLOC], f32)
            deglt = const.tile([128, TLOC], f32)
            nc.sync.dma_start(out=deglt[:],
                              in_=degl_d.ap().rearrange("(t p) -> p t", p=128))
            nc.scalar.activation(out=dinvl[:], in_=deglt[:], func=AF.Sqrt)
            nc.vector.reciprocal(out=dinvl[:], in_=dinvl[:])

            # ---- phase 1: table1 = dinv * (x @ W1), full graph ----
            for t in range(NT):
                xt = work.tile([IN, 128], f32, tag="xt")
                nc.sync.dma_start(out=xt[:],
                                  in_=xT_d.ap()[:, t * 128:(t + 1) * 128])
                ps = ppool.tile([128, H], f32, tag="post")
                nc.tensor.matmul(out=ps[:], lhsT=xt[:],
                                 rhs=W1f[:], start=True, stop=True)
                sb = work.tile([128, H], bf, tag="tb_sb")
                nc.scalar.activation(out=sb[:], in_=ps[:], func=AF.Copy,
                                     scale=dinv[:, t:t + 1])
                nc.scalar.dma_start(out=table1_d.ap()[t * 128:(t + 1) * 128, :],
                                     in_=sb[:])

            # local table1 rows (self-loop contributions), kept in SBUF
            t1loc = const.tile([128, TLOC * H], bf)
            for t in range(TLOC):
                xt = work.tile([IN, 128], f32, tag="xt")
                nc.sync.dma_start(out=xt[:],
                                  in_=xTl_d.ap()[:, t * 128:(t + 1) * 128])
                ps = ppool.tile([128, H], f32, tag="post")
                nc.tensor.matmul(out=ps[:], lhsT=xt[:],
                                 rhs=W1f[:], start=True, stop=True)
                nc.scalar.activation(out=t1loc[:, t * H:(t + 1) * H], in_=ps[:],
                                     func=AF.Copy, scale=dinvl[:, t:t + 1])

            # ---- generic aggregation pass ----
            def agg_pass(idx_dram, dl_tile, tableA_ap, tableB_ap, self_rows,
                         post):
                """for each local dst tile: psum = sum over edges of
                table[src]; then post(t, psum)."""
                call_no = [0]
                for t in range(TLOC):
                    psum = ppool.tile([128, H], f32, tag="agg")
                    chunk0 = t * (CAP_T // 128)
                    nch = CAP_T // 128
                    ci = 0
                    off = t * CAP_T
                    for half, calls in ((0, CALLS_A), (1, CALLS_B)):
                        src_ap = tableA_ap if half == 0 else tableB_ap
                        for n in calls:
                            it = ipool.tile([128, n // 16], i16, tag="idx")
                            nc.sync.dma_start(
                                out=it[:],
                                in_=idx_dram.ap()[:, off // 16:(off + n) // 16])
                            g = gpool.tile([128, n // 128, H], bf, tag="g")
                            if call_no[0] < 6:
                                nc.vector.memset(g[:], 0.0)
                            call_no[0] += 1
                            nc.gpsimd.dma_gather(
                                out_ap=g[:], in_ap=src_ap, idxs_ap=it[:],
                                num_idxs=n, num_idxs_reg=n, elem_size=H,
                                queue_num=(t + ci) % 4)
                            for j in range(n // 128):
                                ind = ipool.tile([128, 128], bf, tag="ind")
                                nc.vector.tensor_tensor(
                                    out=ind[:],
                                    in0=dl_tile[:, chunk0 + ci:chunk0 + ci + 1]
                                        .to_broadcast([128, 128]),
                                    in1=iota_t[:], op=ALU.is_equal)
                                nc.tensor.matmul(
                                    out=psum[:], lhsT=ind[:], rhs=g[:, j, :],
                                    start=(ci == 0), stop=(ci == nch - 1))
                                ci += 1
                            off += n
                    post(t, psum, self_rows(t))

            # ---- phase 2: layer-1 aggregation -> table2 shard ----
            t2loc = const.tile([128, TLOC * H], bf)

            def l1_post(t, psum, selfr):
                s1 = work.tile([128, H], f32, tag="s1")
                nc.vector.tensor_tensor(out=s1[:], in0=psum[:], in1=selfr,
                                        op=ALU.add)
                nc.vector.tensor_tensor(
                    out=s1[:], in0=s1[:],
                    in1=dinvl[:, t:t + 1].to_broadcast([128, H]), op=ALU.mult)
                nc.vector.tensor_tensor(out=s1[:], in0=s1[:], in1=b1t[:],
                                        op=ALU.add)
                nc.scalar.activation(out=t2loc[:, t * H:(t + 1) * H], in_=s1[:],
                                     func=AF.Relu, scale=dinvl[:, t:t + 1])
                nc.scalar.dma_start(out=shard_d.ap()[t * 128:(t + 1) * 128, :],
                                      in_=t2loc[:, t * H:(t + 1) * H])

            agg_pass(idx1_d, dl1,
                     table1_d.ap()[0:HALF, :], table1_d.ap()[HALF:NFULL, :],
                     lambda t: t1loc[:, t * H:(t + 1) * H], l1_post)

            # ---- phase 3: allgather table2 ----
            nc.gpsimd.collective_compute(
                "AllGather", ALU.bypass,
                replica_groups=[list(range(N_CORES))],
                ins=[shard_d.ap()], outs=[t2full_d.ap()])

            # ---- phase 4: layer-2 aggregation -> u2 (node-major) -> u2_fm ----
            u2fm = const.tile([128, NP], bf)

            def l2_post(t, psum, selfr):
                s1 = work.tile([128, H], bf, tag="s2")
                nc.vector.tensor_tensor(out=s1[:], in0=psum[:], in1=selfr,
                                        op=ALU.add)
                nc.vector.tensor_tensor(
                    out=s1[:], in0=s1[:],
                    in1=dinvl[:, t:t + 1].to_broadcast([128, H]), op=ALU.mult)
                ptr = ppool.tile([128, 128], bf, tag="ptb")
                nc.tensor.transpose(out=ptr[:], in_=s1[:], identity=idb[:])
                nc.vector.tensor_copy(out=u2fm[:, t * 128:(t + 1) * 128],
                                      in_=ptr[:])

            agg_pass(idx2_d, dl2,
                     t2full_d.ap()[0:HALF, :], t2full_d.ap()[HALF:FULL2, :],
                     lambda t: t2loc[:, t * H:(t + 1) * H], l2_post)

            # ---- phase 5: h2 = relu(u2 @ W2 + b2) (feature-major) ----
            h2fm = const.tile([128, NP], bf)
            for k in range(NP // 512):
                ps = ppool.tile([128, 512], f32, tag="post")
                nc.tensor.matmul(out=ps[:], lhsT=W2b[:],
                                 rhs=u2fm[:, k * 512:(k + 1) * 512],
                                 start=True, stop=True)
                nc.scalar.activation(out=h2fm[:, k * 512:(k + 1) * 512],
                                     in_=ps[:], func=AF.Relu, bias=b2c[:])

            # ---- phase 6: transpose h2 back, pool via segment matmul ----
            h2nm = const.tile([128, NP], bf)
            for t in range(TLOC):
                ptr = ppool.tile([128, 128], bf, tag="ptb")
                nc.tensor.transpose(out=ptr[:],
                                    in_=h2fm[:, t * 128:(t + 1) * 128],
                                    identity=idb[:])
                nc.vector.tensor_copy(out=h2nm[:, t * 128:(t + 1) * 128],
                                      in_=ptr[:])

            pps = ppool.tile([GPC, H], f32, tag="pool")
            for t in range(TLOC):
                nc.tensor.matmul(out=pps[:],
                                 lhsT=seg_t[:, t * GPC:(t + 1) * GPC],
                                 rhs=h2nm[:, t * 128:(t + 1) * 128],
                                 start=(t == 0), stop=(t == TLOC - 1))
            rc = work.tile([GPC, 1], f32, tag="rc")
            nc.vector.reciprocal(out=rc[:], in_=cnts[:])
            poolnm = work.tile([GPC, H], f32, tag="poolnm")
            nc.scalar.activation(out=poolnm[:], in_=pps[:], func=AF.Copy,
                                 scale=rc[:])
            ptr = ppool.tile([128, GPC], f32, tag="post")
            nc.tensor.transpose(out=ptr[:, :GPC], in_=poolnm[:],
                                identity=idf[:GPC, :GPC])
            poolfm = work.tile([H, GPC], f32, tag="poolfm")
            nc.vector.tensor_copy(out=poolfm[:], in_=ptr[:, :GPC])

            # ---- phase 7: heads ----
            psl = ppool.tile([1, GPC], f32, tag="post")
            nc.tensor.matmul(out=psl[:], lhsT=Wlab[:], rhs=poolfm[:],
                             start=True, stop=True)
            labt = work.tile([1, GPC], f32, tag="lab")
            nc.scalar.activation(out=labt[:], in_=psl[:], func=AF.Sigmoid,
                                 bias=blabc[:])
            nc.sync.dma_start(out=lab_d.ap(), in_=labt[:])

            psd1 = ppool.tile([64, GPC], f32, tag="post")
            nc.tensor.matmul(out=psd1[:], lhsT=Wd1[:], rhs=poolfm[:],
                             start=True, stop=True)
            d1t = work.tile([64, GPC], f32, tag="d1")
            nc.scalar.activation(out=d1t[:], in_=psd1[:], func=AF.Relu,
                                 bias=bd1c[:])
            psd2 = ppool.tile([2, GPC], f32, tag="post")
            nc.tensor.matmul(out=psd2[:], lhsT=Wd2[:], rhs=d1t[:],
                             start=True, stop=True)
            domt = work.tile([2, GPC], f32, tag="dm")
            nc.scalar.activation(out=domt[:], in_=psd2[:], func=AF.Identity,
                                 bias=bd2c[:])
            nc.sync.dma_start(out=dom_d.ap(), in_=domt[:])

    nc.compile()
    return nc


def _prep_inputs(x, edge_index, batch, W1, b1, W2, b2, Wlab, blab, Wd1, bd1,
                 Wd2, bd2, sim_pads=True):
    """Host-side sharding/layout: pure index bucketing + layout transforms."""
    x = np.asarray(x, np.float32)
    ei = np.asarray(edge_index, np.int64)
    batch = np.asarray(batch, np.int64)

    # graph -> core, node ranges (batch sorted)
    nlo = np.searchsorted(batch, np.arange(0, G, GPC))
    nhi = np.append(nlo[1:], N)
    owner = np.zeros(N, np.int64)
    for c in range(N_CORES):
        owner[nlo[c]:nhi[c]] = c
    assert (nhi - nlo).max() <= NP, f"core node count {(nhi-nlo).max()} > {NP}"

    src = np.concatenate([ei[0], np.arange(N, dtype=np.int64)])
    dst = np.concatenate([ei[1], np.arange(N, dtype=np.int64)])
    is_self = np.zeros(len(src), bool)
    is_self[E:] = True

    # degrees at dst (incl self-loops) -- sharding metadata
    deg = np.bincount(dst, minlength=N).astype(np.float32)
    deg_full = np.ones(NFULL, np.float32)
    deg_full[:N] = deg

    # drop self-loops from the gathered stream (handled via local row adds)
    m = ~is_self
    src, dst = src[m], dst[m]

    row2 = owner[src] * NP + (src - nlo[owner[src]])  # layer-2 table row ids

    per_core = []
    for c in range(N_CORES):
        sel = (dst >= nlo[c]) & (dst < nhi[c])
        s_g = src[sel]
        d_l = (dst[sel] - nlo[c]).astype(np.int64)
        r2 = row2[sel]
        order = np.argsort(d_l, kind="stable")
        s_g, d_l, r2 = s_g[order], d_l[order], r2[order]

        def build_stream(rows):
            idx_stream = np.zeros(ECAP, np.int16) if sim_pads else \
                np.full(ECAP, -1, np.int16)
            dl_stream = np.full(ECAP, -1.0, np.float32)
            tile_of = d_l // 128
            for t in range(TLOC):
                smask = tile_of == t
                rt = rows[smask]
                dt_ = d_l[smask] - t * 128
                a = rt < HALF
                ra, da = rt[a], dt_[a]
                rb, db = rt[~a] - HALF, dt_[~a]
                assert len(ra) <= CAP_A, f"tile {t} A overflow {len(ra)}"
                assert len(rb) <= CAP_B, f"tile {t} B overflow {len(rb)}"
                o = t * CAP_T
                idx_stream[o:o + len(ra)] = ra.astype(np.int16)
                dl_stream[o:o + len(ra)] = da
                o2 = o + CAP_A
                idx_stream[o2:o2 + len(rb)] = rb.astype(np.int16)
                dl_stream[o2:o2 + len(rb)] = db
            idxp = np.tile(idx_stream.reshape(ST, 16).T, (8, 1)).copy()
            dlp = np.ascontiguousarray(
                dl_stream.reshape(NCHUNK, 128).T).astype(bf16)
            return idxp, dlp

        idx1, dl1 = build_stream(s_g)
        idx2, dl2 = build_stream(r2)

        nodes_c = nhi[c] - nlo[c]
        xTl = np.zeros((IN, NP), np.float32)
        xTl[:, :nodes_c] = x[nlo[c]:nhi[c]].T
        degl = np.ones(NP, np.float32)
        degl[:nodes_c] = deg[nlo[c]:nhi[c]]

        segind = np.zeros((128, TLOC * GPC), bf16)
        bl = batch[nlo[c]:nhi[c]] - c * GPC
        node_ids = np.arange(nodes_c)
        segind_full = np.zeros((NP, GPC), np.float32)
        segind_full[node_ids, bl] = 1.0
        for t in range(TLOC):
            segind[:, t * GPC:(t + 1) * GPC] = \
                segind_full[t * 128:(t + 1) * 128].astype(bf16)
        cnts = np.maximum(
            np.bincount(bl, minlength=GPC), 1).astype(np.float32)[:, None]

        per_core.append(dict(
            xTl=xTl, degl=degl, idx1=idx1, idx2=idx2, dl1=dl1, dl2=dl2,
            segind=segind, cnts=cnts))

    xT = np.zeros((IN, NFULL), np.float32)
    xT[:, :N] = x.T
    shared = dict(
        xT=xT, deg=deg_full,
        W1i=np.asarray(W1, np.float32), W2i=np.asarray(W2, np.float32),
        Wlabi=np.asarray(Wlab, np.float32),
        Wd1i=np.asarray(Wd1, np.float32), Wd2i=np.asarray(Wd2, np.float32),
        b1t=np.tile(np.asarray(b1, np.float32)[None, :], (128, 1)),
        b2c=np.asarray(b2, np.float32)[:, None],
        blabc=np.asarray(blab, np.float32)[:, None],
        bd1c=np.asarray(bd1, np.float32)[:, None],
        bd2c=np.asarray(bd2, np.float32)[:, None],
        iota=np.tile(np.arange(128, dtype=np.float32)[None, :],
                     (128, 1)).astype(bf16),
    )
    in_maps = [{**shared, **pc} for pc in per_core]
    return in_maps


def kernel(x, edge_index, batch, W1, b1, W2, b2, Wlab, blab, Wd1, bd1, Wd2,
           bd2):
    if "nc" not in _cache:
        _cache["nc"] = _build_module()
    nc = _cache["nc"]
    in_maps = _prep_inputs(x, edge_index, batch, W1, b1, W2, b2, Wlab, blab,
                           Wd1, bd1, Wd2, bd2)
    from concourse.bass_utils import run_bass_kernel_spmd
    res = run_bass_kernel_spmd(nc, in_maps, list(range(N_CORES)))
    labs, doms = [], []
    for c in range(N_CORES):
        labs.append(np.asarray(res.results[c]["lab"]).T)       # [64,1]
        doms.append(np.asarray(res.results[c]["dom"]).T)       # [64,2]
    label_out = np.concatenate(labs, axis=0).astype(np.float32)
    dom_out = np.concatenate(doms, axis=0).astype(np.float32)
    return label_out, dom_out
